# revision 26
# baseline (speedup 1.0000x reference)
"""DLPCNN loss (retrieval-kNN) on 8 Trainium2 NeuronCores via Bass/Tile.

Strategy (data-parallel over the batch, class-sorted):
  - Host sorts rows by class; each of the 8 cores owns 256 contiguous sorted
    rows and a 128-aligned column window (wcol cols) covering the full class
    spans of its rows -- all valid same-class neighbors live in the window.
  - One augmented bf16 matmul per core produces
      NM[i,j] = 2*G - sq_i - sq_j - BIG*(1 - same_class)
    (larger = nearer; cross-class pushed to ~-BIG). The sq rows are carried
    as bf16 hi/lo splits in extra contraction rows so their precision is
    ~fp24 despite the bf16 operand format; accumulation is fp32 in PSUM.
  - Per row: 21st-largest of NM (self included; self is always the row max)
    via 3x (DVE max8 + match_replace); threshold -> 0/1 selection matrix A.
  - W' = A @ [x_w | split(sq_w)] gives neighbor-sum s' and neighbor-sq sum.
  - Identities reduce the loss to per-row scalars:
      g'   = (SNM + (K+1) sq_i + ssq') / 2
      lp_i = sq_i - (2/K)(g' - sq_i) + (||s'||^2 - 2 g' + sq_i)/K^2
  - Device outputs per-row stats (SNM, ssq' parts, ||s'||^2 parts, CE
    max/sumexp); host does the O(B) scalar reduction of the loss terms.

Inputs are pre-swizzled on host to partition-major [128, ktile, cols] so
each DMA is 128 large contiguous descriptors (HWDGE trigger cost is per
descriptor; a k-major layout serializes the whole load on the trigger
engine). rt/lt stream in k-interleaved groups so mm1 k=0 starts ~2us in;
xa streams after them on the same queue (FIFO) to not steal bandwidth
from the mm1-pacing loads.
"""

import sys

for _p in ("/opt/trn_rl_repo",):
    if _p not in sys.path:
        sys.path.insert(0, _p)

import numpy as np
import ml_dtypes

import concourse.bacc as bacc
import concourse.mybir as mybir
import concourse.tile as tile
from concourse.tile import add_dep_helper
from concourse.bass_utils import run_bass_kernel_spmd

B, D, C, K = 2048, 2000, 7, 20
LAMDA = 0.003
NCORES = 8
RPC = B // NCORES          # rows per core
MT = RPC // 128            # m-tiles per core
KR = 2048                  # augmented contraction rows (D data + 12 aug + pad)
KT = KR // 128
NA = D + 2                 # xa columns: [x | sq_hi | sq_lo]
BIG = 65536.0
NEG_FILL = -1.0e30

F32 = mybir.dt.float32
BF16 = mybir.dt.bfloat16
Alu = mybir.AluOpType
Act = mybir.ActivationFunctionType
Ax = mybir.AxisListType

NPBF = ml_dtypes.bfloat16

_CACHE = {}

KGROUPS = [(0, 1), (1, 4), (4, 9), (9, 16)]


def _chunks(total, step=512):
    return [(s, min(step, total - s)) for s in range(0, total, step)]


def _bf_split(v, levels):
    """Split float64 vector v into `levels` bf16 parts summing to ~v."""
    parts = []
    rem = v.astype(np.float64)
    for _ in range(levels):
        p = rem.astype(NPBF)
        parts.append(p)
        rem = rem - p.astype(np.float64)
    return parts


def _build(wcol):
    wt = wcol // 128
    nc = bacc.Bacc("TRN2", target_bir_lowering=False, debug=False)
    lt_d = nc.dram_tensor("lt", [128, KT, RPC], BF16, kind="ExternalInput").ap()
    rt_d = nc.dram_tensor("rt", [128, KT, wcol], BF16, kind="ExternalInput").ap()
    xa_d = nc.dram_tensor("xa", [128, wt, NA], BF16, kind="ExternalInput").ap()
    id_d = nc.dram_tensor("idt", [128, 128], BF16, kind="ExternalInput").ap()
    pm_d = nc.dram_tensor("pm", [128, MT, C], F32, kind="ExternalInput").ap()
    out_d = nc.dram_tensor("out", [128, 20], F32, kind="ExternalOutput").ap()

    with tile.TileContext(nc) as tc:
        with (
            tc.tile_pool(name="data", bufs=1) as data,
            tc.tile_pool(name="work", bufs=2) as work,
            tc.tile_pool(name="small", bufs=1) as small,
            tc.tile_pool(name="pnm", bufs=2, space="PSUM") as pnm,
            tc.tile_pool(name="pw", bufs=1, space="PSUM") as pw,
        ):
            # rt/lt stream in k-groups (small first group -> early mm1 start);
            # idt/pm early (tiny); xa last on the same queue (FIFO priority)
            lt = data.tile([128, KT, RPC], BF16)
            rt = data.tile([128, KT, wcol], BF16)
            # one queue, xa last: the serial trigger stream doubles as a
            # priority order -- xa only starts streaming once the mm1-pacing
            # rt/lt groups are queued
            for (a, b) in KGROUPS:
                nc.sync.dma_start(lt[:, a:b], lt_d[:, a:b])
                nc.sync.dma_start(rt[:, a:b], rt_d[:, a:b])
            idt = small.tile([128, 128], BF16)
            nc.sync.dma_start(idt[:], id_d[:])
            pmt = small.tile([128, MT, C], F32)
            nc.sync.dma_start(pmt[:], pm_d[:])
            xa = data.tile([128, wt, NA], BF16)
            for (a, b) in [(0, wt // 2), (wt // 2, wt)]:
                nc.sync.dma_start(xa[:, a:b], xa_d[:, a:b])

            outb = small.tile([128, 20], F32)
            atb = small.tile([128, wt, RPC], BF16)   # A^T (bf16)

            # ---- CE pieces (independent; fills engine idle at start) ----
            for m in range(MT):
                nc.vector.reduce_max(outb[:, 16 + m:17 + m], pmt[:, m, :], axis=Ax.X)
                negmx = work.tile([128, 1], F32)
                nc.vector.tensor_scalar_mul(negmx[:], outb[:, 16 + m:17 + m], -1.0)
                e7 = work.tile([128, C], F32)
                nc.scalar.activation(
                    e7[:], pmt[:, m, :], Act.Exp, bias=negmx[:, 0:1], scale=1.0,
                    accum_out=outb[:, 18 + m:19 + m],
                )

            # ---- NM = 2G - sq_i - sq_j - BIG*(1-same) ----
            # both m-tiles' matmuls are emitted BEFORE any top-k consumer:
            # engine streams are executed in program order, so this keeps PE
            # grinding mm1(m1) while DVE runs m0's top-k chain
            nms = []
            for m in range(MT):
                ms = slice(m * 128, (m + 1) * 128)
                nm = pnm.tile([128, wcol], F32, tag="nm", bufs=2, name=f"nm{m}")
                nms.append(nm)
                for (s, n) in _chunks(wcol):
                    for k in range(KT):
                        nc.tensor.matmul(
                            nm[:, s:s + n],
                            lhsT=lt[:, k, ms],
                            rhs=rt[:, k, s:s + n],
                            start=(k == 0),
                            stop=(k == KT - 1),
                        )

            mnch = []
            v3s = []
            cch = _chunks(wcol)
            ncc = len(cch)
            for m in range(MT):
                ms = slice(m * 128, (m + 1) * 128)
                nm = nms[m]

                # ---- top-(K+1) threshold, chunked: each chunk's top-24 is
                # extracted as soon as that chunk's PSUM accumulation ends
                # (so it hides under the next chunk's k-sweep on PE), then a
                # cheap [128, 24*ncc] merge finds the row-global 21st ----
                chunks_m = []
                cc = work.tile([128, 24 * ncc], F32, tag="cc", name=f"cc{m}")
                for ci, (s, n) in enumerate(cch):
                    mnc = work.tile([128, n], F32, tag=f"mn{ci}", name=f"mn{m}_{ci}")
                    nc.vector.tensor_copy(mnc[:], nm[:, s:s + n])
                    chunks_m.append(mnc)
                    o = 24 * ci
                    nc.vector.max(cc[:, o:o + 8], mnc[:])
                    m2 = work.tile([128, n], F32, tag=f"mn2_{ci}",
                                   name=f"mn2_{m}_{ci}")
                    nc.vector.match_replace(m2[:], cc[:, o:o + 8], mnc[:], NEG_FILL)
                    nc.vector.max(cc[:, o + 8:o + 16], m2[:])
                    m3 = work.tile([128, n], F32, tag=f"mn3_{ci}",
                                   name=f"mn3_{m}_{ci}")
                    nc.vector.match_replace(m3[:], cc[:, o + 8:o + 16], m2[:],
                                            NEG_FILL)
                    nc.vector.max(cc[:, o + 16:o + 24], m3[:])
                mnch.append(chunks_m)

                v1 = work.tile([128, 8], F32)
                nc.vector.max(v1[:], cc[:])
                cc2 = work.tile([128, 24 * ncc], F32, tag="cc2", name=f"cc2{m}")
                nc.vector.match_replace(cc2[:], v1[:], cc[:], NEG_FILL)
                v2 = work.tile([128, 8], F32)
                nc.vector.max(v2[:], cc2[:])
                cc3 = work.tile([128, 24 * ncc], F32, tag="cc3", name=f"cc3{m}")
                nc.vector.match_replace(cc3[:], v2[:], cc2[:], NEG_FILL)
                v3 = work.tile([128, 8], F32)
                nc.vector.max(v3[:], cc3[:])
                v3s.append(v3)

                # A = (NM >= t) as bf16 first (unblocks PE transposes early)
                abh = work.tile([128, wcol], BF16)
                for ci, (s, n) in enumerate(cch):
                    nc.vector.tensor_scalar(abh[:, s:s + n], chunks_m[ci][:],
                                            v3[:, 4:5], None, op0=Alu.is_ge)
                for t in range(wt):
                    tr = pnm.tile([128, 128], BF16, tag="nm", bufs=2, name=f"tr{m}_{t}")
                    nc.tensor.transpose(tr[:], abh[:, t * 128:(t + 1) * 128], idt[:])
                    if t % 2 == 0:
                        nc.vector.tensor_copy(atb[:, t, ms], tr[:])
                    else:
                        nc.scalar.copy(atb[:, t, ms], tr[:])

                # ---- W' = A @ [x_w | sq_hi | sq_lo] ----
                # one single-bank PSUM tile per 512-chunk so each chunk's
                # matmul group is independent of the others' square-reduces
                for ci, (s, n) in enumerate(_chunks(NA)):
                    w = pw.tile([128, n], F32, tag=f"w{ci}", name=f"w{m}_{ci}")
                    for t in range(wt):
                        nc.tensor.matmul(
                            w[:],
                            lhsT=atb[:, t, ms],
                            rhs=xa[:, t, s:s + n],
                            start=(t == 0),
                            stop=(t == wt - 1),
                        )
                    # pipelined ||s'||^2: square-reduce each chunk as soon as
                    # its accumulation group completes (exclude the sq cols)
                    ne = min(s + n, D) - s
                    sq2 = work.tile([128, 512], BF16, tag="sq2")
                    nc.scalar.activation(
                        sq2[:, :ne], w[:, :ne], Act.Square,
                        accum_out=outb[:, 8 + 4 * m + ci:9 + 4 * m + ci],
                    )
                    if s + n > D:
                        lo = D - s
                        if m == 0:
                            nc.scalar.copy(outb[:, 4 + m:5 + m], w[:, lo:lo + 1])
                            nc.scalar.copy(outb[:, 6 + m:7 + m], w[:, lo + 1:lo + 2])
                        else:
                            nc.vector.tensor_copy(outb[:, 4 + m:5 + m], w[:, lo:lo + 1])
                            nc.vector.tensor_copy(outb[:, 6 + m:7 + m], w[:, lo + 1:lo + 2])

            # deferred SNM reduces (off the critical top-k chain), chunked;
            # host sums the per-chunk partials
            for m in range(MT):
                for ci, (s, n) in enumerate(cch):
                    scr = work.tile([128, n], F32, tag=f"scr{ci}",
                                    name=f"scr{m}_{ci}")
                    nc.vector.scalar_tensor_tensor(
                        out=scr[:], in0=mnch[m][ci][:], scalar=v3s[m][:, 4:5],
                        in1=mnch[m][ci][:],
                        op0=Alu.is_ge, op1=Alu.mult,
                        accum_out=outb[:, 2 * m + ci:2 * m + ci + 1],
                    )

            nc.sync.dma_start(out_d[:], outb[:])

    nc.compile()
    return nc


def _plan_windows(ys):
    starts_c = np.searchsorted(ys, np.arange(C))
    ends_c = np.searchsorted(ys, np.arange(C), side="right")
    need = []
    for c in range(NCORES):
        blo, bhi = c * RPC, (c + 1) * RPC
        cls = np.unique(ys[blo:bhi])
        lo = int(min(starts_c[k] for k in cls))
        hi = int(max(ends_c[k] for k in cls))
        need.append((lo, hi))
    wneed = max(hi - (lo // 128) * 128 for lo, hi in need)
    wcol = 128 * ((wneed + 127) // 128)
    wcol = max(wcol, 512)
    starts = []
    for (lo, hi) in need:
        ws = (lo // 128) * 128
        ws = min(ws, B - wcol)
        assert ws + wcol >= hi and ws <= lo
        starts.append(ws)
    return wcol, starts


def kernel(preds, x, y):
    y = np.asarray(y).astype(np.int64)
    preds = np.ascontiguousarray(np.asarray(preds, dtype=np.float32))
    x = np.ascontiguousarray(np.asarray(x, dtype=np.float32))
    assert x.shape == (B, D) and preds.shape == (B, C) and y.shape == (B,)

    order = np.argsort(y, kind="stable")
    xs = x[order]
    ys = y[order]
    ps = preds[order]
    sq64 = np.einsum("ij,ij->i", xs.astype(np.float64), xs.astype(np.float64))
    sq = sq64.astype(np.float32)

    wcol, starts = _plan_windows(ys)
    cls_count = np.bincount(ys, minlength=C)
    assert (cls_count >= K + 1).all(), cls_count

    oh = np.zeros((C, B), np.float32)
    oh[ys, np.arange(B)] = 1.0

    # global augmented rhs for NM matmul [KR, B] in bf16:
    #   rows 0..D-1: x^T ; D..D+2: split(-(sq+BIG)) with lhsT ones
    #   D+3..D+9: one-hot(class) with lhsT BIG*one-hot ;
    #   D+10..D+11: ones with lhsT split(-sq_i) ; rest zero
    rhs_g = np.zeros((KR, B), NPBF)
    rhs_g[:D] = xs.T.astype(NPBF)
    r1, r2, r3 = _bf_split(-(sq64 + BIG), 3)
    rhs_g[D], rhs_g[D + 1], rhs_g[D + 2] = r1, r2, r3
    one = np.float32(1.0)
    rhs_g[D + 3:D + 3 + C] = oh.astype(NPBF)
    rhs_g[D + 10] = one
    rhs_g[D + 11] = one
    # partition-major swizzle [KR, B] -> [128, KT, B]
    rhs_gp = np.ascontiguousarray(rhs_g.reshape(KT, 128, B).transpose(1, 0, 2))

    xa_g = np.zeros((B, NA), NPBF)
    xa_g[:, :D] = xs.astype(NPBF)
    q1, q2 = _bf_split(sq64, 2)
    xa_g[:, D] = q1
    xa_g[:, D + 1] = q2

    if wcol not in _CACHE:
        _CACHE[wcol] = _build(wcol)
    nc = _CACHE[wcol]
    wt = wcol // 128

    in_maps = []
    for cidx in range(NCORES):
        my = slice(cidx * RPC, (cidx + 1) * RPC)
        ws = starts[cidx]
        lhsT = np.zeros((KR, RPC), NPBF)
        lhsT[:D] = (2.0 * xs[my].T).astype(NPBF)
        s1, s2 = _bf_split(-sq64[my], 2)
        lhsT[D + 10] = s1
        lhsT[D + 11] = s2
        lhsT[D] = one
        lhsT[D + 1] = one
        lhsT[D + 2] = one
        lhsT[D + 3:D + 3 + C] = (BIG * oh[:, my]).astype(NPBF)
        in_maps.append({
            "lt": np.ascontiguousarray(lhsT.reshape(KT, 128, RPC).transpose(1, 0, 2)),
            "rt": np.ascontiguousarray(rhs_gp[:, :, ws:ws + wcol]),
            "xa": np.ascontiguousarray(
                xa_g[ws:ws + wcol].reshape(wt, 128, NA).transpose(1, 0, 2)),
            "idt": np.eye(128, dtype=NPBF),
            "pm": np.ascontiguousarray(
                ps[my].reshape(MT, 128, C).transpose(1, 0, 2)),
        })

    res = run_bass_kernel_spmd(nc, in_maps, core_ids=list(range(NCORES)))

    # host-side unshard: per-row stats -> two scalar loss terms
    lp_sum = 0.0
    ce_sum = 0.0
    for cidx in range(NCORES):
        my = slice(cidx * RPC, (cidx + 1) * RPC)
        o = res.results[cidx]["out"].astype(np.float64)
        snm = np.stack([o[:, 0:2].sum(1), o[:, 2:4].sum(1)]).reshape(RPC)
        ssq = (o[:, 4:6] + o[:, 6:8]).T.reshape(RPC)
        ssn = np.stack([o[:, 8:12].sum(1), o[:, 12:16].sum(1)]).reshape(RPC)
        mx = o[:, 16:18].T.reshape(RPC)
        se = o[:, 18:20].T.reshape(RPC)
        sq_my = sq[my].astype(np.float64)
        gp = 0.5 * (snm + (K + 1) * sq_my + ssq)
        lp = sq_my - (2.0 / K) * (gp - sq_my) + (ssn - 2.0 * gp + sq_my) / K**2
        lp_sum += lp.sum()
        lse = np.log(se) + mx
        pick = ps[my][np.arange(RPC), ys[my]].astype(np.float64)
        ce_sum += (lse - pick).sum()

    loss = LAMDA * (lp_sum / B) / 2.0 + ce_sum / B
    return np.float32(loss)


# revision 30
# speedup vs baseline: 1.0279x; 1.0279x over previous
"""DLPCNN loss (retrieval-kNN) on 8 Trainium2 NeuronCores via Bass/Tile.

Strategy (data-parallel over the batch, class-sorted):
  - Host sorts rows by class; each of the 8 cores owns 256 contiguous sorted
    rows and a 128-aligned column window (wcol cols) covering the full class
    spans of its rows -- all valid same-class neighbors live in the window.
  - One augmented bf16 matmul per core produces
      NM[i,j] = 2*G - sq_i - sq_j - BIG*(1 - same_class)
    (larger = nearer; cross-class pushed to ~-BIG). The sq rows are carried
    as bf16 hi/lo splits in extra contraction rows so their precision is
    ~fp24 despite the bf16 operand format; accumulation is fp32 in PSUM.
  - Per row: 21st-largest of NM (self included; self is always the row max)
    via 3x (DVE max8 + match_replace); threshold -> 0/1 selection matrix A.
  - W' = A @ [x_w | split(sq_w)] gives neighbor-sum s' and neighbor-sq sum.
  - Identities reduce the loss to per-row scalars:
      g'   = (SNM + (K+1) sq_i + ssq') / 2
      lp_i = sq_i - (2/K)(g' - sq_i) + (||s'||^2 - 2 g' + sq_i)/K^2
  - Device outputs per-row stats (SNM, ssq' parts, ||s'||^2 parts, CE
    max/sumexp); host does the O(B) scalar reduction of the loss terms.

Inputs are pre-swizzled on host to partition-major [128, ktile, cols] so
each DMA is 128 large contiguous descriptors (HWDGE trigger cost is per
descriptor; a k-major layout serializes the whole load on the trigger
engine). rt/lt stream in k-interleaved groups so mm1 k=0 starts ~2us in;
xa streams after them on the same queue (FIFO) to not steal bandwidth
from the mm1-pacing loads.
"""

import sys

for _p in ("/opt/trn_rl_repo",):
    if _p not in sys.path:
        sys.path.insert(0, _p)

import numpy as np
import ml_dtypes

import concourse.bacc as bacc
import concourse.mybir as mybir
import concourse.tile as tile
from concourse.tile import add_dep_helper
from concourse.bass_utils import run_bass_kernel_spmd

B, D, C, K = 2048, 2000, 7, 20
LAMDA = 0.003
NCORES = 8
RPC = B // NCORES          # rows per core
MT = RPC // 128            # m-tiles per core
KR = 2048                  # augmented contraction rows (D data + 12 aug + pad)
KT = KR // 128
NA = D + 2                 # xa columns: [x | sq_hi | sq_lo]
BIG = 65536.0
NEG_FILL = -1.0e30

F32 = mybir.dt.float32
BF16 = mybir.dt.bfloat16
Alu = mybir.AluOpType
Act = mybir.ActivationFunctionType
Ax = mybir.AxisListType

NPBF = ml_dtypes.bfloat16

_CACHE = {}

KGROUPS = [(0, 1), (1, 3), (3, 6), (6, 10), (10, 13), (13, 16)]


def _chunks(total, step=512):
    return [(s, min(step, total - s)) for s in range(0, total, step)]


def _bf_split(v, levels):
    """Split float64 vector v into `levels` bf16 parts summing to ~v."""
    parts = []
    rem = v.astype(np.float64)
    for _ in range(levels):
        p = rem.astype(NPBF)
        parts.append(p)
        rem = rem - p.astype(np.float64)
    return parts


def _build(wcol):
    wt = wcol // 128
    nc = bacc.Bacc("TRN2", target_bir_lowering=False, debug=False)
    lt_d = nc.dram_tensor("lt", [128, KT, RPC], BF16, kind="ExternalInput").ap()
    rt_d = nc.dram_tensor("rt", [128, KT, wcol], BF16, kind="ExternalInput").ap()
    xa_d = nc.dram_tensor("xa", [128, wt, NA], BF16, kind="ExternalInput").ap()
    id_d = nc.dram_tensor("idt", [128, 128], BF16, kind="ExternalInput").ap()
    pm_d = nc.dram_tensor("pm", [128, MT, C], F32, kind="ExternalInput").ap()
    out_d = nc.dram_tensor("out", [128, 20], F32, kind="ExternalOutput").ap()

    with tile.TileContext(nc) as tc:
        with (
            tc.tile_pool(name="data", bufs=1) as data,
            tc.tile_pool(name="work", bufs=2) as work,
            tc.tile_pool(name="small", bufs=1) as small,
            tc.tile_pool(name="pnm", bufs=2, space="PSUM") as pnm,
            tc.tile_pool(name="pw", bufs=1, space="PSUM") as pw,
        ):
            # rt/lt stream in k-groups (small first group -> early mm1 start);
            # idt/pm early (tiny); xa last on the same queue (FIFO priority)
            lt = data.tile([128, KT, RPC], BF16)
            rt = data.tile([128, KT, wcol], BF16)
            # one queue, xa last: the serial trigger stream doubles as a
            # priority order -- xa only starts streaming once the mm1-pacing
            # rt/lt groups are queued
            for (a, b) in KGROUPS:
                nc.sync.dma_start(lt[:, a:b], lt_d[:, a:b])
                nc.sync.dma_start(rt[:, a:b], rt_d[:, a:b])
            idt = small.tile([128, 128], BF16)
            nc.sync.dma_start(idt[:], id_d[:])
            pmt = small.tile([128, MT, C], F32)
            nc.sync.dma_start(pmt[:], pm_d[:])
            xa = data.tile([128, wt, NA], BF16)
            for (a, b) in [(0, wt // 2), (wt // 2, wt)]:
                nc.sync.dma_start(xa[:, a:b], xa_d[:, a:b])

            outb = small.tile([128, 20], F32)
            atb = small.tile([128, wt, RPC], BF16)   # A^T (bf16)

            # ---- CE pieces (independent; fills engine idle at start) ----
            for m in range(MT):
                nc.vector.reduce_max(outb[:, 16 + m:17 + m], pmt[:, m, :], axis=Ax.X)
                negmx = work.tile([128, 1], F32)
                nc.vector.tensor_scalar_mul(negmx[:], outb[:, 16 + m:17 + m], -1.0)
                e7 = work.tile([128, C], F32)
                nc.scalar.activation(
                    e7[:], pmt[:, m, :], Act.Exp, bias=negmx[:, 0:1], scale=1.0,
                    accum_out=outb[:, 18 + m:19 + m],
                )

            # ---- NM = 2G - sq_i - sq_j - BIG*(1-same) ----
            # both m-tiles' matmuls are emitted BEFORE any top-k consumer:
            # engine streams are executed in program order, so this keeps PE
            # grinding mm1(m1) while DVE runs m0's top-k chain
            nms = []
            for m in range(MT):
                ms = slice(m * 128, (m + 1) * 128)
                nm = pnm.tile([128, wcol], F32, tag="nm", bufs=2, name=f"nm{m}")
                nms.append(nm)
                for (s, n) in _chunks(wcol):
                    for k in range(KT):
                        nc.tensor.matmul(
                            nm[:, s:s + n],
                            lhsT=lt[:, k, ms],
                            rhs=rt[:, k, s:s + n],
                            start=(k == 0),
                            stop=(k == KT - 1),
                        )

            mns = []
            v3s = []
            for m in range(MT):
                ms = slice(m * 128, (m + 1) * 128)
                nm = nms[m]

                # single PSUM read; everything downstream reads the SBUF copy
                mn = work.tile([128, wcol], F32)
                nc.vector.tensor_copy(mn[:], nm[:])
                mns.append(mn)

                # ---- top-(K+1) threshold: 3 rounds of max8 ----
                v1 = work.tile([128, 8], F32)
                nc.vector.max(v1[:], mn[:])
                mn2 = work.tile([128, wcol], F32)
                nc.vector.match_replace(mn2[:], v1[:], mn[:], NEG_FILL)
                v2 = work.tile([128, 8], F32)
                nc.vector.max(v2[:], mn2[:])
                mn3 = work.tile([128, wcol], F32)
                nc.vector.match_replace(mn3[:], v2[:], mn2[:], NEG_FILL)
                v3 = work.tile([128, 8], F32)
                nc.vector.max(v3[:], mn3[:])
                v3s.append(v3)

                # A = (NM >= t) as bf16 first (unblocks PE transposes early)
                abh = work.tile([128, wcol], BF16)
                nc.vector.tensor_scalar(abh[:], mn[:], v3[:, 4:5], None, op0=Alu.is_ge)
                for t in range(wt):
                    tr = pnm.tile([128, 128], BF16, tag="nm", bufs=2, name=f"tr{m}_{t}")
                    nc.tensor.transpose(tr[:], abh[:, t * 128:(t + 1) * 128], idt[:])
                    if t % 2 == 0:
                        nc.vector.tensor_copy(atb[:, t, ms], tr[:])
                    else:
                        nc.scalar.copy(atb[:, t, ms], tr[:])

                # ---- W' = A @ [x_w | sq_hi | sq_lo] ----
                # one single-bank PSUM tile per 512-chunk so each chunk's
                # matmul group is independent of the others' square-reduces
                for ci, (s, n) in enumerate(_chunks(NA)):
                    w = pw.tile([128, n], F32, tag=f"w{ci}", name=f"w{m}_{ci}")
                    for t in range(wt):
                        nc.tensor.matmul(
                            w[:],
                            lhsT=atb[:, t, ms],
                            rhs=xa[:, t, s:s + n],
                            start=(t == 0),
                            stop=(t == wt - 1),
                        )
                    # pipelined ||s'||^2: square-reduce each chunk as soon as
                    # its accumulation group completes (exclude the sq cols)
                    ne = min(s + n, D) - s
                    sq2 = work.tile([128, 512], BF16, tag="sq2")
                    nc.scalar.activation(
                        sq2[:, :ne], w[:, :ne], Act.Square,
                        accum_out=outb[:, 8 + 4 * m + ci:9 + 4 * m + ci],
                    )
                    if s + n > D:
                        lo = D - s
                        if m == 0:
                            nc.scalar.copy(outb[:, 4 + m:5 + m], w[:, lo:lo + 1])
                            nc.scalar.copy(outb[:, 6 + m:7 + m], w[:, lo + 1:lo + 2])
                        else:
                            nc.vector.tensor_copy(outb[:, 4 + m:5 + m], w[:, lo:lo + 1])
                            nc.vector.tensor_copy(outb[:, 6 + m:7 + m], w[:, lo + 1:lo + 2])

            # deferred SNM reduces (off the critical top-k chain)
            for m in range(MT):
                scr = work.tile([128, wcol], F32)
                nc.vector.scalar_tensor_tensor(
                    out=scr[:], in0=mns[m][:], scalar=v3s[m][:, 4:5],
                    in1=mns[m][:],
                    op0=Alu.is_ge, op1=Alu.mult,
                    accum_out=outb[:, 2 * m:2 * m + 1],
                )

            nc.sync.dma_start(out_d[:], outb[:])

    nc.compile()
    return nc


def _plan_windows(ys):
    starts_c = np.searchsorted(ys, np.arange(C))
    ends_c = np.searchsorted(ys, np.arange(C), side="right")
    need = []
    for c in range(NCORES):
        blo, bhi = c * RPC, (c + 1) * RPC
        cls = np.unique(ys[blo:bhi])
        lo = int(min(starts_c[k] for k in cls))
        hi = int(max(ends_c[k] for k in cls))
        need.append((lo, hi))
    wneed = max(hi - (lo // 128) * 128 for lo, hi in need)
    wcol = 128 * ((wneed + 127) // 128)
    wcol = max(wcol, 512)
    starts = []
    for (lo, hi) in need:
        ws = (lo // 128) * 128
        ws = min(ws, B - wcol)
        assert ws + wcol >= hi and ws <= lo
        starts.append(ws)
    return wcol, starts


def kernel(preds, x, y):
    y = np.asarray(y).astype(np.int64)
    preds = np.ascontiguousarray(np.asarray(preds, dtype=np.float32))
    x = np.ascontiguousarray(np.asarray(x, dtype=np.float32))
    assert x.shape == (B, D) and preds.shape == (B, C) and y.shape == (B,)

    order = np.argsort(y, kind="stable")
    xs = x[order]
    ys = y[order]
    ps = preds[order]
    sq64 = np.einsum("ij,ij->i", xs.astype(np.float64), xs.astype(np.float64))
    sq = sq64.astype(np.float32)

    wcol, starts = _plan_windows(ys)
    cls_count = np.bincount(ys, minlength=C)
    assert (cls_count >= K + 1).all(), cls_count

    oh = np.zeros((C, B), np.float32)
    oh[ys, np.arange(B)] = 1.0

    # global augmented rhs for NM matmul [KR, B] in bf16:
    #   rows 0..D-1: x^T ; D..D+2: split(-(sq+BIG)) with lhsT ones
    #   D+3..D+9: one-hot(class) with lhsT BIG*one-hot ;
    #   D+10..D+11: ones with lhsT split(-sq_i) ; rest zero
    rhs_g = np.zeros((KR, B), NPBF)
    rhs_g[:D] = xs.T.astype(NPBF)
    r1, r2, r3 = _bf_split(-(sq64 + BIG), 3)
    rhs_g[D], rhs_g[D + 1], rhs_g[D + 2] = r1, r2, r3
    one = np.float32(1.0)
    rhs_g[D + 3:D + 3 + C] = oh.astype(NPBF)
    rhs_g[D + 10] = one
    rhs_g[D + 11] = one
    # partition-major swizzle [KR, B] -> [128, KT, B]
    rhs_gp = np.ascontiguousarray(rhs_g.reshape(KT, 128, B).transpose(1, 0, 2))

    xa_g = np.zeros((B, NA), NPBF)
    xa_g[:, :D] = xs.astype(NPBF)
    q1, q2 = _bf_split(sq64, 2)
    xa_g[:, D] = q1
    xa_g[:, D + 1] = q2

    if wcol not in _CACHE:
        _CACHE[wcol] = _build(wcol)
    nc = _CACHE[wcol]
    wt = wcol // 128

    in_maps = []
    for cidx in range(NCORES):
        my = slice(cidx * RPC, (cidx + 1) * RPC)
        ws = starts[cidx]
        lhsT = np.zeros((KR, RPC), NPBF)
        lhsT[:D] = (2.0 * xs[my].T).astype(NPBF)
        s1, s2 = _bf_split(-sq64[my], 2)
        lhsT[D + 10] = s1
        lhsT[D + 11] = s2
        lhsT[D] = one
        lhsT[D + 1] = one
        lhsT[D + 2] = one
        lhsT[D + 3:D + 3 + C] = (BIG * oh[:, my]).astype(NPBF)
        in_maps.append({
            "lt": np.ascontiguousarray(lhsT.reshape(KT, 128, RPC).transpose(1, 0, 2)),
            "rt": np.ascontiguousarray(rhs_gp[:, :, ws:ws + wcol]),
            "xa": np.ascontiguousarray(
                xa_g[ws:ws + wcol].reshape(wt, 128, NA).transpose(1, 0, 2)),
            "idt": np.eye(128, dtype=NPBF),
            "pm": np.ascontiguousarray(
                ps[my].reshape(MT, 128, C).transpose(1, 0, 2)),
        })

    res = run_bass_kernel_spmd(nc, in_maps, core_ids=list(range(NCORES)))

    # host-side unshard: per-row stats -> two scalar loss terms
    lp_sum = 0.0
    ce_sum = 0.0
    for cidx in range(NCORES):
        my = slice(cidx * RPC, (cidx + 1) * RPC)
        o = res.results[cidx]["out"].astype(np.float64)
        snm = np.stack([o[:, 0], o[:, 2]]).reshape(RPC)
        ssq = (o[:, 4:6] + o[:, 6:8]).T.reshape(RPC)
        ssn = np.stack([o[:, 8:12].sum(1), o[:, 12:16].sum(1)]).reshape(RPC)
        mx = o[:, 16:18].T.reshape(RPC)
        se = o[:, 18:20].T.reshape(RPC)
        sq_my = sq[my].astype(np.float64)
        gp = 0.5 * (snm + (K + 1) * sq_my + ssq)
        lp = sq_my - (2.0 / K) * (gp - sq_my) + (ssn - 2.0 * gp + sq_my) / K**2
        lp_sum += lp.sum()
        lse = np.log(se) + mx
        pick = ps[my][np.arange(RPC), ys[my]].astype(np.float64)
        ce_sum += (lse - pick).sum()

    loss = LAMDA * (lp_sum / B) / 2.0 + ce_sum / B
    return np.float32(loss)
